# revision 30
# baseline (speedup 1.0000x reference)
"""Trainium2 Bass kernel for nn_CAM_6949257085456.

Pure data-parallel over batch: 8 cores x 64 samples. v2 redesign focused on
DMA-issue cost and PE density:

  - All DRAM->SBUF traffic is pre-tiled on the host so each transfer is one
    big 2D DMA (14 KB/partition lines for x). ~60 dma_starts/core instead of
    ~550 (sync-sequencer DIRECT2D issue is ~585 ns each, fixed).
  - The vis GEMM streams 196 chunk matmuls back-to-back per sub-batch
    (supertile-granular deps) so the PE can ramp to full p-state.
  - The branch (attention) stage avoids SBUF scatter DMAs entirely:
    per-sample PE transposes write direct to 32-aligned PSUM offsets, the
    16x16 per-sample affine is one block-diagonal matmul per 4 samples, and
    elementwise work is spread over Scalar/DVE/GpSimd.
  - Branch work of sub-batch b-1 is interleaved between the supertiles of
    sub-batch b's stream, so the PE stays busy during DMA-bound stretches.

Host-side algebraic folds (exact in fp32):
  - vis path: X @ W_red.T @ W_enc2.T == X @ (W_enc2 @ W_red).T
  - regressors have no nonlinearity: feats@Wv1.T@Wv2.T == feats @ (Wv2@Wv1).T
Everything fed to the chip is bf16 (fp32 PSUM accumulation).
"""
import sys

if "/opt/trn_rl_repo" not in sys.path:
    sys.path.insert(0, "/opt/trn_rl_repo")

import numpy as np
import ml_dtypes

import concourse.bacc as bacc
import concourse.bass as bass
import concourse.mybir as mybir
import concourse.tile as tile
from concourse import bass_utils

BF16 = mybir.dt.bfloat16
F32 = mybir.dt.float32
AF = mybir.ActivationFunctionType

B, T, DA, DVFULL, DH = 512, 16, 512, 25088, 128
NCORES = 8
SCALE = 1.0 / 16.0  # 1/sqrt(256)

_CACHE = {}


class Dims:
    """Geometry for one core. G chunks of 128 per supertile."""

    def __init__(self, DV, S, NSB=2, G=14):
        self.DV = DV
        self.KC = DV // 128           # contraction chunks
        self.G = G                    # chunks per supertile
        self.NST = self.KC // G       # supertiles
        assert self.NST * G == self.KC
        self.S = S                    # samples per core
        self.NSB = NSB                # sub-batches
        self.SBS = S // NSB           # samples per sub-batch
        self.RB = self.SBS * T        # rows per sub-batch
        assert self.RB <= 512
        self.TILES = self.SBS // 4    # avf tiles per sub-batch (4 samples ea)
        self.R = S * T                # rows per core


def build_graph(tc, io, D: Dims):
    """Emit the whole per-core program. io maps tensor name -> DRAM AP."""
    import os

    STAGE = int(os.environ.get("KSTAGE", "3"))
    nc = tc.nc
    from contextlib import ExitStack

    with ExitStack() as stack:
        ec = stack.enter_context
        cpool = ec(tc.tile_pool(name="const", bufs=1))
        wpool = ec(tc.tile_pool(name="wred", bufs=D.NST))
        xpool = ec(tc.tile_pool(name="xin", bufs=5))
        actpool = ec(tc.tile_pool(name="acts", bufs=2 * D.NSB))
        avfpool = ec(tc.tile_pool(name="avf", bufs=4))
        gpool = ec(tc.tile_pool(name="gsb", bufs=3))
        att4pool = ec(tc.tile_pool(name="att4", bufs=2))
        htsbpool = ec(tc.tile_pool(name="htsb", bufs=2))
        outsbpool = ec(tc.tile_pool(name="outsb", bufs=2 * D.NSB))
        finpool = ec(tc.tile_pool(name="fin", bufs=2))
        # PSUM: exactly 8 banks
        encpool = ec(tc.tile_pool(name="enc_ps", bufs=2, space="PSUM"))
        attpool = ec(tc.tile_pool(name="att_ps", bufs=2, space="PSUM"))
        htpool = ec(tc.tile_pool(name="ht_ps", bufs=1, space="PSUM"))
        scrpool = ec(tc.tile_pool(name="scr_ps", bufs=2, space="PSUM"))

        # ---- hot-path constants first: aud encoder can start immediately ----
        f1_sb = cpool.tile([128, 4, D.R], BF16, name="f1sb")
        nc.sync.dma_start(f1_sb[:], io["f1t"])
        wenc1_sb = cpool.tile([128, 4, DH], BF16, name="wenc1")
        nc.sync.dma_start(wenc1_sb[:], io["wenc1"])
        b1_sb = cpool.tile([DH, 1], F32, name="b1sb")
        nc.sync.dma_start(b1_sb[:], io["b1"])
        b2_sb = cpool.tile([DH, 1], F32, name="b2sb")
        nc.sync.dma_start(b2_sb[:], io["b2"])

        # branch-stage constants: issued lazily, early in sub-batch 0's stream
        ident_sb = cpool.tile([128, 128], BF16, name="ident")
        wblka_sb = cpool.tile([128, 128], BF16, name="wblka")
        wblkv_sb = cpool.tile([128, 128], BF16, name="wblkv")
        wa_sb = cpool.tile([128, 32], BF16, name="wasb")
        wca_sb = cpool.tile([128, 2, 32], BF16, name="wcasb")
        wh_sb = cpool.tile([32, 32], BF16, name="whsb")
        wrega_sb = cpool.tile([128, 2], BF16, name="wrega")
        wregv_sb = cpool.tile([128, 2], BF16, name="wregv")
        creg_sb = cpool.tile([2, 1], F32, name="cregsb")

        def issue_branch_consts():
            nc.sync.dma_start(ident_sb[:], io["ident"])
            nc.sync.dma_start(wblka_sb[:], io["wblk_a"])
            nc.sync.dma_start(wblkv_sb[:], io["wblk_v"])
            nc.sync.dma_start(wa_sb[:], io["waT"])
            nc.sync.dma_start(wca_sb[:], io["wcaT"])
            nc.sync.dma_start(wh_sb[:], io["whT"])
            nc.sync.dma_start(wrega_sb[:], io["wreg_a"])
            nc.sync.dma_start(wregv_sb[:], io["wreg_v"])
            nc.sync.dma_start(creg_sb[:], io["creg"])

        # weight supertiles: tiles declared here, DMA issued just-in-time
        # inside sub-batch 0's stream loop (paired with its xg issues)
        wt_tiles = []
        for i in range(D.NST):
            wt = wpool.tile([128, D.G, 128], BF16, tag="wt", name=f"wt{i}")
            wt_tiles.append(wt)

        wblk = {0: wblka_sb, 1: wblkv_sb}

        def emit_tile_front(ctx, m):
            """transposes + avf + A(blockdiag) + att matmuls + tanh."""
            audT, visT = ctx["audT"], ctx["visT"]
            sb = ctx["b"]
            tr = scrpool.tile([128, 1024], BF16, tag="scr", name=f"tr{sb}_{m}")
            trf = scrpool.tile([16, 1024], BF16, tag="scr", name=f"trf{sb}_{m}")
            for q in range(4):
                s = 4 * m + q
                nc.tensor.transpose(
                    tr[32 * q : 32 * q + 16, 0:128],
                    audT[:, 16 * s : 16 * s + 16],
                    ident_sb[:],
                    tile_position=(0, 32 * q),
                )
                nc.tensor.transpose(
                    tr[32 * q : 32 * q + 16, 128:256],
                    visT[:, 16 * s : 16 * s + 16],
                    ident_sb[:],
                    tile_position=(0, 32 * q),
                )
                nc.tensor.transpose(
                    trf[0:16, 128 * q : 128 * q + 128],
                    audT[:, 16 * s : 16 * s + 16],
                    ident_sb[:],
                )
                nc.tensor.transpose(
                    trf[0:16, 512 + 128 * q : 512 + 128 * q + 128],
                    visT[:, 16 * s : 16 * s + 16],
                    ident_sb[:],
                )
            avf = avfpool.tile([128, 256], BF16, tag="avf", name=f"avf{sb}_{m}")
            nc.gpsimd.memset(avf[:], 0.0)
            for q in range(4):
                nc.vector.tensor_copy(
                    avf[32 * q : 32 * q + 16, :], tr[32 * q : 32 * q + 16, 0:256]
                )
            avff = avfpool.tile(
                [16, 1024], BF16, tag="avff", name=f"aff{sb}_{m}", bufs=2
            )
            nc.vector.tensor_copy(avff[:], trf[:])
            a4 = scrpool.tile([128, 512], F32, tag="scr", name=f"a4{sb}_{m}")
            for bi in range(2):
                nc.tensor.matmul(
                    a4[:, 128 * bi : 128 * bi + 128],
                    wblk[bi][:],
                    avf[:, 128 * bi : 128 * bi + 128],
                    start=True,
                    stop=True,
                )
            g = gpool.tile([128, 256], BF16, tag="g", name=f"g{sb}_{m}")
            nc.vector.tensor_copy(g[:], a4[:, 0:256])
            att4 = att4pool.tile([128, 2048], BF16, tag="att4", name=f"at4{sb}_{m}")
            for q in range(4):
                attps = attpool.tile([128, 512], F32, tag="att", name=f"ap{sb}_{m}{q}")
                for jh in range(2):
                    nc.tensor.matmul(
                        attps[:, 256 * jh : 256 * jh + 256],
                        avf[32 * q : 32 * q + 16, 128 * jh : 128 * jh + 128],
                        g[32 * q : 32 * q + 16, :],
                        start=True,
                        stop=True,
                        tile_position=(32 * q, 0),
                    )
                nc.scalar.activation(
                    att4[:, 512 * q : 512 * q + 512], attps[:], AF.Tanh, scale=SCALE
                )
            ctx["avf"][m] = avf
            ctx["avff"][m] = avff
            ctx["att4"][m] = att4

        def emit_tile_back(ctx, m):
            """ht = relu(Wca@att + Wa@fts), out = Wh@ht, residual adds."""
            audT, visT = ctx["audT"], ctx["visT"]
            sb = ctx["b"]
            avf = ctx["avf"].pop(m)
            avff = ctx["avff"].pop(m)
            att4 = ctx["att4"].pop(m)
            use_fts = STAGE >= 23 or STAGE == 3
            use_dve_relu = STAGE >= 24 or STAGE == 3
            htps = htpool.tile([32, 1024], F32, tag="ht", name=f"ht{sb}_{m}")
            for bi in range(2):
                if use_fts:
                    nc.tensor.matmul(
                        htps[0:32, 512 * bi : 512 * bi + 512],
                        wa_sb[0:16, :],
                        avff[0:16, 512 * bi : 512 * bi + 512],
                        start=True,
                        stop=False,
                    )
                att4v = att4[:].rearrange("p (q u) -> p q u", u=512)
                for jh in range(2):
                    c0 = 256 * jh + 128 * bi
                    nc.tensor.matmul(
                        htps[0:32, 512 * bi : 512 * bi + 512],
                        wca_sb[:, jh, :],
                        att4v[:, :, c0 : c0 + 128],
                        start=(jh == 0 and not use_fts),
                        stop=(jh == 1),
                    )
            htsb = htsbpool.tile([32, 1024], BF16, tag="htsb", name=f"hs{sb}_{m}")
            nc.scalar.activation(htsb[0:32, 0:512], htps[0:32, 0:512], AF.Relu)
            if use_dve_relu:
                nc.vector.tensor_relu(htsb[0:32, 512:1024], htps[0:32, 512:1024])
            else:
                nc.scalar.activation(htsb[0:32, 512:1024], htps[0:32, 512:1024], AF.Relu)
            outp = scrpool.tile([128, 512], F32, tag="scr", name=f"op{sb}_{m}")
            for bi in range(2):
                for q in range(4):
                    nc.tensor.matmul(
                        outp[:, 64 * bi + 16 * q : 64 * bi + 16 * q + 16],
                        htsb[0:32, 512 * bi + 128 * q : 512 * bi + 128 * q + 128],
                        wh_sb[0:32, 16 * bi : 16 * bi + 16],
                        start=True,
                        stop=True,
                    )
            nc.vector.tensor_add(
                ctx["outa"][:, 64 * m : 64 * m + 64],
                outp[:, 0:64],
                audT[:, 64 * m : 64 * m + 64],
            )
            nc.vector.tensor_add(
                ctx["outv"][:, 64 * m : 64 * m + 64],
                outp[:, 64:128],
                visT[:, 64 * m : 64 * m + 64],
            )

        def emit_regressor(ctx):
            sb = ctx["b"]
            regps = attpool.tile([2, 512], F32, tag="att", name=f"reg{sb}")
            nc.tensor.matmul(
                regps[:, 0 : D.RB],
                wrega_sb[:],
                ctx["outa"][:, 0 : D.RB],
                start=True,
                stop=False,
            )
            nc.tensor.matmul(
                regps[:, 0 : D.RB],
                wregv_sb[:],
                ctx["outv"][:, 0 : D.RB],
                start=False,
                stop=True,
            )
            fin = finpool.tile([2, 512], F32, tag="fin", name=f"fin{sb}")
            nc.scalar.activation(
                fin[:, 0 : D.RB], regps[:, 0 : D.RB], AF.Identity, bias=creg_sb[:]
            )
            nc.sync.dma_start(io["vouts"][sb], fin[0:1, 0 : D.RB])
            nc.sync.dma_start(io["aouts"][sb], fin[1:2, 0 : D.RB])

        def emit_tile_stub(ctx, m):
            """STAGE<3 debug: bypass branch math, outa/outv = audT/visT."""
            nc.vector.tensor_copy(
                ctx["outa"][:, 64 * m : 64 * m + 64],
                ctx["audT"][:, 64 * m : 64 * m + 64],
            )
            nc.vector.tensor_copy(
                ctx["outv"][:, 64 * m : 64 * m + 64],
                ctx["visT"][:, 64 * m : 64 * m + 64],
            )

        def make_units(ctx):
            """Software-pipelined order: front(m+1) before back(m) so the
            tanh chain of tile m hides behind tile m+1's PE work."""
            fronts, backs = [], []
            for m in range(D.TILES):
                if STAGE >= 3:
                    fronts.append(lambda m=m: emit_tile_front(ctx, m))
                    backs.append(lambda m=m: emit_tile_back(ctx, m))
                elif STAGE == 2:
                    fronts.append(lambda m=m: emit_tile_front(ctx, m))
                    backs.append(lambda m=m: emit_tile_stub(ctx, m))
                else:
                    fronts.append(lambda m=m: emit_tile_stub(ctx, m))
            units = []
            if STAGE >= 2:
                for m in range(D.TILES):
                    units.append(fronts[m])
                    if m >= 1:
                        units.append(backs[m - 1])
                if backs:
                    units.append(backs[D.TILES - 1])
            else:
                units = fronts
            units.append(lambda: emit_regressor(ctx))
            return units

        prev_units = []
        ctxs = []
        for b in range(D.NSB):
            ctx = {"b": b, "avf": {}, "avff": {}, "att4": {}}
            ctxs.append(ctx)

            def emit_aud(ctx=ctx, b=b):
                audps = encpool.tile([128, 512], F32, tag="enc", name=f"audps{b}")
                for c in range(4):
                    nc.tensor.matmul(
                        audps[:, 0 : D.RB],
                        wenc1_sb[:, c, :],
                        f1_sb[:, c, b * D.RB : (b + 1) * D.RB],
                        start=(c == 0),
                        stop=(c == 3),
                    )
                audT = actpool.tile([128, 512], BF16, tag="act", name=f"audT{b}")
                nc.scalar.activation(
                    audT[:, 0 : D.RB], audps[:, 0 : D.RB], AF.Identity, bias=b1_sb[:]
                )
                ctx["audT"] = audT

            # vis stream, interleaving branch work of sub-batch b-1
            visps = encpool.tile([128, 512], F32, tag="enc", name=f"visps{b}")
            emitted = 0
            for i in range(D.NST):
                xg = xpool.tile([128, D.G, 512], BF16, tag="xg", name=f"xg{b}_{i}")
                nc.sync.dma_start(xg[:, :, 0 : D.RB], io["xg"][b, i])
                if b == 0:
                    nc.sync.dma_start(wt_tiles[i][:], io["wt"][i])
                    if i == min(4, D.NST - 1):
                        issue_branch_consts()
                for j in range(D.G):
                    k = D.G * i + j
                    nc.tensor.matmul(
                        visps[:, 0 : D.RB],
                        wt_tiles[i][:, j, :],
                        xg[:, j, 0 : D.RB],
                        start=(k == 0),
                        stop=(k == D.KC - 1),
                    )
                if i == min(1, D.NST - 1):
                    emit_aud()
                target = (i + 1) * len(prev_units) // D.NST
                while emitted < target:
                    prev_units[emitted]()
                    emitted += 1
            while emitted < len(prev_units):
                prev_units[emitted]()
                emitted += 1
            visT = actpool.tile([128, 512], BF16, tag="act", name=f"visT{b}")
            nc.scalar.activation(
                visT[:, 0 : D.RB], visps[:, 0 : D.RB], AF.Identity, bias=b2_sb[:]
            )
            ctx["visT"] = visT
            outa = outsbpool.tile([128, 512], BF16, tag="outsb", name=f"oa{b}")
            outv = outsbpool.tile([128, 512], BF16, tag="outsb", name=f"ov{b}")
            ctx["outa"], ctx["outv"] = outa, outv
            prev_units = make_units(ctx)

        # drain the last sub-batch's branch work
        for u in prev_units:
            u()


def _build(D: Dims):
    nc = bacc.Bacc("TRN2", target_bir_lowering=False, debug=False)

    io = {}
    io["xg"] = nc.dram_tensor(
        "xg", [D.NSB, D.NST, 128, D.G, D.RB], BF16, kind="ExternalInput"
    ).ap()
    io["wt"] = nc.dram_tensor(
        "wt", [D.NST, 128, D.G, 128], BF16, kind="ExternalInput"
    ).ap()
    io["f1t"] = nc.dram_tensor("f1t", [128, 4, D.R], BF16, kind="ExternalInput").ap()
    io["wenc1"] = nc.dram_tensor(
        "wenc1", [128, 4, DH], BF16, kind="ExternalInput"
    ).ap()
    for name, shape, dt in [
        ("b1", [DH, 1], F32),
        ("b2", [DH, 1], F32),
        ("wblk_a", [128, 128], BF16),
        ("wblk_v", [128, 128], BF16),
        ("waT", [128, 32], BF16),
        ("wcaT", [128, 2, 32], BF16),
        ("whT", [32, 32], BF16),
        ("wreg_a", [128, 2], BF16),
        ("wreg_v", [128, 2], BF16),
        ("creg", [2, 1], F32),
        ("ident", [128, 128], BF16),
    ]:
        io[name] = nc.dram_tensor(name, shape, dt, kind="ExternalInput").ap()
    io["vouts"] = nc.dram_tensor("vouts", [D.NSB, D.RB], F32, kind="ExternalOutput").ap()
    io["aouts"] = nc.dram_tensor("aouts", [D.NSB, D.RB], F32, kind="ExternalOutput").ap()

    with tile.TileContext(nc) as tc:
        build_graph(tc, io, D)

    nc.compile()
    return nc


def prep_shared(inputs, D: Dims):
    """Host-side folds + weight tiling shared by all cores."""
    f32 = np.float32
    bf = ml_dtypes.bfloat16
    W_enc1 = np.asarray(inputs["W_enc1"], f32)
    W_enc2 = np.asarray(inputs["W_enc2"], f32)
    W_red = np.asarray(inputs["W_red"], f32)
    W2r = W_enc2 @ W_red                                    # [128, DV]
    b2v = W_enc2 @ np.asarray(inputs["b_red"], f32) + np.asarray(inputs["b_enc2"], f32)
    wv = (np.asarray(inputs["Wv2"], f32) @ np.asarray(inputs["Wv1"], f32))[0]
    cv = float((np.asarray(inputs["Wv2"], f32) @ np.asarray(inputs["bv1"], f32)
                + np.asarray(inputs["bv2"], f32))[0])
    wa = (np.asarray(inputs["Wa2"], f32) @ np.asarray(inputs["Wa1"], f32))[0]
    ca = float((np.asarray(inputs["Wa2"], f32) @ np.asarray(inputs["ba1"], f32)
                + np.asarray(inputs["ba2"], f32))[0])

    def blockdiag(W_aff):
        M = np.zeros((128, 128), f32)
        blk = np.asarray(W_aff, f32).T  # [t, o]
        for q in range(4):
            M[32 * q : 32 * q + 16, 32 * q : 32 * q + 16] = blk
        return M

    def padgroups(mat, rows):
        out = np.zeros((128, mat.shape[1]), f32)
        for q in range(4):
            out[32 * q : 32 * q + rows] = mat
        return out

    # wt[i, p, j, f] = W2r.T[(G*i+j)*128+p, f]
    wt = (
        np.ascontiguousarray(W2r.T)
        .reshape(D.NST, D.G, 128, 128)
        .transpose(0, 2, 1, 3)
    )
    wenc1t = W_enc1.T.reshape(4, 128, DH).transpose(1, 0, 2)

    wh = np.zeros((32, 32), f32)
    wh[:, 0:16] = np.asarray(inputs["W_ha"], f32).T
    wh[:, 16:32] = np.asarray(inputs["W_hv"], f32).T

    shared = {
        "wt": np.ascontiguousarray(wt).astype(bf),
        "wenc1": np.ascontiguousarray(wenc1t).astype(bf),
        "b1": np.asarray(inputs["b_enc1"], f32).reshape(DH, 1),
        "b2": b2v.reshape(DH, 1),
        "wblk_a": blockdiag(inputs["W_affa"]).astype(bf),
        "wblk_v": blockdiag(inputs["W_affv"]).astype(bf),
        "waT": padgroups(np.asarray(inputs["W_a"], f32).T, 16).astype(bf),
        "wcaT": np.asarray(inputs["W_ca"], f32)
        .T.reshape(2, 128, 32)
        .transpose(1, 0, 2)
        .astype(bf)
        .copy(),
        "whT": wh.astype(bf),
        "wreg_a": np.stack([wv[:128], wa[:128]], 1).astype(bf),
        "wreg_v": np.stack([wv[128:], wa[128:]], 1).astype(bf),
        "creg": np.array([[cv], [ca]], f32),
        "ident": np.eye(128, dtype=f32).astype(bf),
    }
    return shared


def prep_core(f1_core, f2_core, D: Dims):
    """Per-core activation tiling. f1_core [R, DA], f2_core [R, DV] fp32."""
    bf = ml_dtypes.bfloat16
    # xg[b, i, p, j, r] = f2[b*RB + r, (G*i+j)*128 + p]
    xg = (
        f2_core.reshape(D.NSB, D.RB, D.NST, D.G, 128)
        .transpose(0, 2, 4, 3, 1)
    )
    # f1t[p, c, r] = f1[r, c*128+p]
    f1t = f1_core.reshape(D.R, 4, 128).transpose(2, 1, 0)
    return {
        "xg": np.ascontiguousarray(xg).astype(bf),
        "f1t": np.ascontiguousarray(f1t).astype(bf),
    }


def kernel(**inputs):
    D = Dims(DV=DVFULL, S=B // NCORES)
    if "nc" not in _CACHE:
        _CACHE["nc"] = _build(D)
    nc = _CACHE["nc"]

    shared = prep_shared(inputs, D)
    f1 = np.asarray(inputs["f1_norm"], np.float32).reshape(B * T, DA)
    f2 = np.asarray(inputs["f2_norm"], np.float32).reshape(B * T, DVFULL)

    in_maps = []
    for c in range(NCORES):
        rs = slice(c * D.R, (c + 1) * D.R)
        m = dict(shared)
        m.update(prep_core(f1[rs], f2[rs], D))
        in_maps.append(m)

    import os

    res = bass_utils.run_bass_kernel_spmd(
        nc,
        in_maps,
        core_ids=list(range(NCORES)),
        trace=bool(os.environ.get("KERNEL_TRACE")),
    )
    _CACHE["last_results"] = res

    S = B // NCORES
    vouts = np.concatenate(
        [r["vouts"].reshape(S, T) for r in res.results], axis=0
    ).astype(np.float32)
    aouts = np.concatenate(
        [r["aouts"].reshape(S, T) for r in res.results], axis=0
    ).astype(np.float32)
    return vouts, aouts
